# revision 1
# baseline (speedup 1.0000x reference)
"""Causal kernel (nn_CausalKernel) for 8x TRN2 NeuronCores.

Algorithm: sum_n k_n sin(n*r) decomposed via n = a*297 + b:
  sin(n r) = sin_a cos_b + cos_a sin_b with
  sin_b = sin(2pi frac(b * r/2pi)), sin_a = sin(2pi frac(a * 297r/2pi)).
Per-point trig tables are built mode-major ([modes, points]) with a
magic-number round chain feeding the ScalarE Sin LUT (valid range [-pi, pi]);
the 35937-mode contraction runs on TensorE in bf16.

Pure data parallel: 8 cores x 16384 points; weights replicated.
"""
import sys
sys.path.insert(0, "/opt/trn_rl_repo")

import numpy as np
import ml_dtypes

import concourse.bass as bass
import concourse.mybir as mybir
import concourse.tile as tile
from concourse.bass_utils import run_bass_kernel_spmd

f32 = np.float32
bf16 = ml_dtypes.bfloat16

N_CORES = 8
NPT = 16384            # points per core
NI = 2048              # points per point-tile
NTILES = NPT // NI     # 8
NCH = 512              # matmul moving-dim chunk (one PSUM bank)
NCHUNKS = NI // NCH    # 4

D1, D2 = 297, 121      # n = a*D1 + b
C1 = 99                # D1 contraction chunk rows (3 chunks)
MT = 33                # temporal modes

MAGIC = float(f32(1.5 * 2 ** 23))
INV2PI = float(f32(1.0 / (2 * np.pi)))
TWO_PI_M = float(f32(6.2831845))   # < 2pi so |scale*0.5| <= pi
PI_HALF = float(f32(np.pi / 2))
DT = mybir.dt


def _build_nc(mass_parameter: float, coupling_strength: float, use_act_offload: bool = True):
    nc = bass.Bass(target_bir_lowering=False)
    AF = mybir.ActivationFunctionType
    OP = mybir.AluOpType

    coords_in = nc.dram_tensor("coords", [NPT, 4], DT.float32, kind="ExternalInput")
    wk_in = nc.dram_tensor("wk", [C1, 3 * D2], DT.bfloat16, kind="ExternalInput")
    sc_in = nc.dram_tensor("sc", [128, 8], DT.float32, kind="ExternalInput")
    tkw_in = nc.dram_tensor("tkw", [MT, 1], DT.bfloat16, kind="ExternalInput")
    out_d = nc.dram_tensor("out", [NPT], DT.float32, kind="ExternalOutput")
    dbg_r = nc.dram_tensor("dbg_r", [128, 128], DT.float32, kind="ExternalOutput")
    dbg_psi = nc.dram_tensor("dbg_psi", [128, 128], DT.float32, kind="ExternalOutput")
    dbg_sp = nc.dram_tensor("dbg_sp", [1, NPT], DT.bfloat16, kind="ExternalOutput")
    dbg_t = nc.dram_tensor("dbg_t", [1, NPT], DT.bfloat16, kind="ExternalOutput")
    bpsi_d = nc.dram_tensor("bpsi", [1, NPT], DT.float32)
    bphi_d = nc.dram_tensor("bphi", [1, NPT], DT.float32)
    btab_d = nc.dram_tensor("btab", [1, NPT], DT.float32)

    mp = float(f32(mass_parameter))
    cs = float(f32(coupling_strength))

    with SafeTileContext(nc) as tc:
        with (
            tc.tile_pool(name="const", bufs=1) as cpool,
            tc.tile_pool(name="pm", bufs=1) as pm,          # point-major persistents
            tc.tile_pool(name="bc", bufs=2) as bc,          # broadcast tiles
            tc.tile_pool(name="chain", bufs=2) as ch,       # chain scratch
            tc.tile_pool(name="tab", bufs=2) as tb,
            tc.tile_pool(name="tab2", bufs=1) as tb2,         # bf16 tables
            tc.tile_pool(name="ps", bufs=2, space="PSUM") as ps,
            tc.tile_pool(name="psr", bufs=1, space="PSUM") as psr,
        ):
            # ---------------- constants ----------------
            sc0 = cpool.tile([128, 8], DT.float32)
            nc.sync.dma_start(sc0[:], sc_in[:])
            sc = cpool.tile([128, 8], DT.float32)
            nc.vector.tensor_copy(out=sc[:], in_=sc0[:])    # absorb DMA sem on DVE

            wk0 = cpool.tile([C1, 3 * D2], DT.bfloat16)
            nc.sync.dma_start(wk0[:], wk_in[:])
            wk = cpool.tile([C1, 3 * D2], DT.bfloat16)
            nc.vector.tensor_copy(out=wk[:], in_=wk0[:])

            tkw0 = cpool.tile([MT, 1], DT.bfloat16)
            nc.sync.dma_start(tkw0[:], tkw_in[:])
            tkw = cpool.tile([MT, 1], DT.bfloat16)
            nc.vector.tensor_copy(out=tkw[:], in_=tkw0[:])

            ones121 = cpool.tile([D2, 1], DT.bfloat16)
            nc.vector.memset(ones121[:], 1.0)
            pi_half_t = cpool.tile([128, 1], DT.float32)
            nc.vector.memset(pi_half_t[:], PI_HALF)
            magic_t = cpool.tile([128, 1], DT.float32)
            nc.vector.memset(magic_t[:], MAGIC)
            nmagic_t = cpool.tile([128, 1], DT.float32)
            nc.vector.memset(nmagic_t[:], -MAGIC)

            # ---------------- stage 0: point-major precompute ----------------
            crd = pm.tile([128, 512], DT.float32)
            nc.sync.dma_start(crd[:], coords_in.rearrange("(p f) c -> p (f c)", p=128))
            crd4 = crd[:].rearrange("p (f c) -> p f c", c=4)

            t_pm = pm.tile([128, 128], DT.float32)
            nc.vector.tensor_copy(out=t_pm[:], in_=crd4[:, :, 0])
            xx = pm.tile([128, 128], DT.float32, tag="w1")
            yy = pm.tile([128, 128], DT.float32, tag="w2")
            zz = pm.tile([128, 128], DT.float32, tag="w3")
            nc.vector.tensor_mul(out=xx[:], in0=crd4[:, :, 1], in1=crd4[:, :, 1])
            nc.vector.tensor_mul(out=yy[:], in0=crd4[:, :, 2], in1=crd4[:, :, 2])
            nc.vector.tensor_mul(out=zz[:], in0=crd4[:, :, 3], in1=crd4[:, :, 3])
            sdsq = pm.tile([128, 128], DT.float32)
            nc.vector.tensor_add(out=sdsq[:], in0=xx[:], in1=yy[:])
            nc.vector.tensor_add(out=sdsq[:], in0=sdsq[:], in1=zz[:])
            r2e = pm.tile([128, 128], DT.float32)
            nc.vector.tensor_scalar_add(out=r2e[:], in0=sdsq[:], scalar1=float(f32(1e-12)))

            # r = sqrt(r2e) with two Newton refinements (HW sqrt LUT is loose)
            r_pm = pm.tile([128, 128], DT.float32)
            nc.scalar.activation(out=r_pm[:], in_=r2e[:], func=AF.Sqrt)
            tmpa = pm.tile([128, 128], DT.float32, tag="w1")
            tmpb = pm.tile([128, 128], DT.float32, tag="w2")
            for _ in range(2):
                nc.vector.reciprocal(out=tmpa[:], in_=r_pm[:])
                nc.vector.tensor_mul(out=tmpb[:], in0=r2e[:], in1=tmpa[:])
                nc.vector.tensor_add(out=tmpb[:], in0=tmpb[:], in1=r_pm[:])
                nc.vector.tensor_scalar_mul(out=r_pm[:], in0=tmpb[:], scalar1=0.5)

            # psi1 = frac(r/2pi), signed
            A0 = pm.tile([128, 128], DT.float32)
            m0 = pm.tile([128, 128], DT.float32)
            psi1 = pm.tile([128, 128], DT.float32)
            nc.vector.tensor_scalar(out=A0[:], in0=r_pm[:], scalar1=INV2PI,
                                    scalar2=MAGIC, op0=OP.mult, op1=OP.add)
            nc.vector.tensor_scalar_add(out=m0[:], in0=A0[:], scalar1=-MAGIC)
            nc.vector.scalar_tensor_tensor(out=psi1[:], in0=r_pm[:], scalar=INV2PI,
                                           in1=m0[:], op0=OP.mult, op1=OP.subtract)

            # phi1 = frac(D1 * r / 2pi) via 12-bit split of r (accuracy for a<=120 amplification)
            SC12 = float(f32(2.0 ** 12))
            c2_64 = np.float64(D1) / (2 * np.pi)
            c2h = float(f32(np.trunc(c2_64 * 2 ** 12) / 2 ** 12))
            c2l = float(f32(c2_64 - np.float64(f32(c2h))))
            c2f = float(f32(c2_64))
            rh = pm.tile([128, 128], DT.float32)
            rl = pm.tile([128, 128], DT.float32)
            nc.vector.tensor_scalar(out=A0[:], in0=r_pm[:], scalar1=SC12,
                                    scalar2=MAGIC, op0=OP.mult, op1=OP.add)
            nc.vector.tensor_scalar_add(out=m0[:], in0=A0[:], scalar1=-MAGIC)
            nc.vector.tensor_scalar_mul(out=rh[:], in0=m0[:], scalar1=float(f32(2.0 ** -12)))
            nc.vector.tensor_sub(out=rl[:], in0=r_pm[:], in1=rh[:])
            # t1 = rh*c2h (exact); f1 = frac(t1)
            t1t = pm.tile([128, 128], DT.float32, tag="w3")
            nc.vector.tensor_scalar(out=A0[:], in0=rh[:], scalar1=c2h,
                                    scalar2=MAGIC, op0=OP.mult, op1=OP.add)
            nc.vector.tensor_scalar_add(out=m0[:], in0=A0[:], scalar1=-MAGIC)
            nc.vector.scalar_tensor_tensor(out=t1t[:], in0=rh[:], scalar=c2h,
                                           in1=m0[:], op0=OP.mult, op1=OP.subtract)
            # rest = rh*c2l + rl*c2 ; ph = f1 + rest ; phi1 = frac(ph)
            nc.vector.tensor_scalar_mul(out=tmpa[:], in0=rl[:], scalar1=c2f)
            nc.vector.scalar_tensor_tensor(out=tmpb[:], in0=rh[:], scalar=c2l,
                                           in1=tmpa[:], op0=OP.mult, op1=OP.add)
            ph_t = pm.tile([128, 128], DT.float32)
            nc.vector.tensor_add(out=ph_t[:], in0=t1t[:], in1=tmpb[:])
            phi1 = pm.tile([128, 128], DT.float32)
            nc.vector.tensor_scalar(out=A0[:], in0=ph_t[:], scalar1=1.0,
                                    scalar2=MAGIC, op0=OP.mult, op1=OP.add)
            nc.vector.tensor_scalar_add(out=m0[:], in0=A0[:], scalar1=-MAGIC)
            nc.vector.tensor_sub(out=phi1[:], in0=ph_t[:], in1=m0[:])

            # |t|, envelope, green, masks, 1/(r+1e-6)
            tabs = pm.tile([128, 128], DT.float32)
            nc.vector.tensor_scalar_mul(out=tabs[:], in0=t_pm[:], scalar1=-1.0)
            nc.vector.tensor_max(out=tabs[:], in0=tabs[:], in1=t_pm[:])
            env_pm = pm.tile([128, 128], DT.float32)
            nc.scalar.activation(out=env_pm[:], in_=tabs[:], func=AF.Exp,
                                 scale=float(f32(-0.1)))
            expg = pm.tile([128, 128], DT.float32)
            nc.scalar.activation(out=expg[:], in_=r_pm[:], func=AF.Exp, scale=-mp)
            rinv = pm.tile([128, 128], DT.float32)
            nc.vector.reciprocal(out=rinv[:], in_=r_pm[:])
            green = pm.tile([128, 128], DT.float32)
            nc.vector.tensor_mul(out=green[:], in0=expg[:], in1=rinv[:])
            nc.vector.tensor_scalar_mul(out=green[:], in0=green[:], scalar1=cs)
            rden = pm.tile([128, 128], DT.float32)
            nc.vector.tensor_scalar_add(out=rden[:], in0=r_pm[:], scalar1=float(f32(1e-6)))
            rdinv = pm.tile([128, 128], DT.float32)
            nc.vector.reciprocal(out=rdinv[:], in_=rden[:])

            tsq = pm.tile([128, 128], DT.float32)
            nc.vector.tensor_mul(out=tsq[:], in0=t_pm[:], in1=t_pm[:])
            interval = pm.tile([128, 128], DT.float32)
            nc.vector.tensor_sub(out=interval[:], in0=tsq[:], in1=sdsq[:])
            mg1 = pm.tile([128, 128], DT.float32, tag="w4")
            mg2 = pm.tile([128, 128], DT.float32, tag="w5")
            nc.vector.tensor_scalar(out=mg1[:], in0=interval[:], scalar1=0.0,
                                    scalar2=None, op0=OP.is_gt)
            nc.vector.tensor_scalar(out=mg2[:], in0=t_pm[:], scalar1=0.0,
                                    scalar2=None, op0=OP.is_gt)
            nc.vector.tensor_mul(out=mg1[:], in0=mg1[:], in1=mg2[:])
            nc.vector.tensor_mul(out=green[:], in0=green[:], in1=mg1[:])
            mo1 = pm.tile([128, 128], DT.float32, tag="w4")
            mo2 = pm.tile([128, 128], DT.float32, tag="w5")
            nc.vector.tensor_scalar(out=mo1[:], in0=interval[:], scalar1=0.0,
                                    scalar2=None, op0=OP.is_ge)
            nc.vector.tensor_scalar(out=mo2[:], in0=t_pm[:], scalar1=0.0,
                                    scalar2=None, op0=OP.is_ge)
            maskout = pm.tile([128, 128], DT.float32)
            nc.vector.tensor_mul(out=maskout[:], in0=mo1[:], in1=mo2[:])

            # bases to DRAM for broadcast-DMA sourcing
            nc.sync.dma_start(bpsi_d[:].rearrange("o (p f) -> (o p) f", p=128), psi1[:])
            nc.sync.dma_start(bphi_d[:].rearrange("o (p f) -> (o p) f", p=128), phi1[:])
            nc.sync.dma_start(btab_d[:].rearrange("o (p f) -> (o p) f", p=128), tabs[:])

            # staging for reduced rows (bf16)
            stg_sp = pm.tile([1, NPT], DT.bfloat16)
            stg_t = pm.tile([1, NPT], DT.bfloat16)

            # ---------------- per point-tile mode-major pipeline ----------------
            for tt_i in range(NTILES):
                pslc = slice(tt_i * NI, (tt_i + 1) * NI)
                b_psi = bc.tile([C1, NI], DT.float32, tag="b_psi")
                b_phi = bc.tile([D2, NI], DT.float32, tag="b_phi")
                b_tab = bc.tile([MT, NI], DT.float32, tag="b_tab")
                nc.gpsimd.dma_start(b_psi[:], bpsi_d[0:1, pslc].to_broadcast((C1, NI)))
                nc.gpsimd.dma_start(b_phi[:], bphi_d[0:1, pslc].to_broadcast((D2, NI)))
                nc.gpsimd.dma_start(b_tab[:], btab_d[0:1, pslc].to_broadcast((MT, NI)))

                # D1 tables: f in [-0.5,0.5]; sin/cos via Sin LUT; bf16 out
                sin1 = tb2.tile([C1, 3 * NI], DT.bfloat16, tag="sin1")
                cos1 = tb2.tile([C1, 3 * NI], DT.bfloat16, tag="cos1")
                for c in range(3):
                    Ac = ch.tile([C1, NI], DT.float32, tag="Ac")
                    fc_ = ch.tile([C1, NI], DT.float32, tag="fc")
                    scal = sc[:C1, c:c + 1]
                    if use_act_offload:
                        nc.scalar.activation(out=Ac[:], in_=b_psi[:], func=AF.Identity,
                                             bias=magic_t[:C1], scale=scal)
                        nc.scalar.activation(out=Ac[:], in_=Ac[:], func=AF.Identity,
                                             bias=nmagic_t[:C1], scale=1.0)
                    else:
                        nc.vector.tensor_scalar(out=Ac[:], in0=b_psi[:], scalar1=scal,
                                                scalar2=MAGIC, op0=OP.mult, op1=OP.add)
                        nc.vector.tensor_scalar_add(out=Ac[:], in0=Ac[:], scalar1=-MAGIC)
                    nc.vector.scalar_tensor_tensor(out=fc_[:], in0=b_psi[:], scalar=scal,
                                                   in1=Ac[:], op0=OP.mult, op1=OP.subtract)
                    nc.scalar.activation(out=sin1[:, c * NI:(c + 1) * NI], in_=fc_[:],
                                         func=AF.Sin, scale=TWO_PI_M)
                    nc.vector.tensor_scalar_mul(out=Ac[:], in0=fc_[:], scalar1=-1.0)
                    nc.vector.tensor_max(out=fc_[:], in0=fc_[:], in1=Ac[:])
                    nc.scalar.activation(out=cos1[:, c * NI:(c + 1) * NI], in_=fc_[:],
                                         func=AF.Sin, scale=-TWO_PI_M, bias=pi_half_t[:C1])

                # D2 tables
                sin2 = tb2.tile([D2, NI], DT.bfloat16, tag="sin2")
                cos2 = tb2.tile([D2, NI], DT.bfloat16, tag="cos2")
                A2 = ch.tile([D2, NI], DT.float32, tag="Ac")
                f2_ = ch.tile([D2, NI], DT.float32, tag="fc")
                scal2 = sc[:D2, 3:4]
                nc.vector.tensor_scalar(out=A2[:], in0=b_phi[:], scalar1=scal2,
                                        scalar2=MAGIC, op0=OP.mult, op1=OP.add)
                nc.vector.tensor_scalar_add(out=A2[:], in0=A2[:], scalar1=-MAGIC)
                nc.vector.scalar_tensor_tensor(out=f2_[:], in0=b_phi[:], scalar=scal2,
                                               in1=A2[:], op0=OP.mult, op1=OP.subtract)
                nc.scalar.activation(out=sin2[:], in_=f2_[:], func=AF.Sin, scale=TWO_PI_M)
                nc.vector.tensor_scalar_mul(out=A2[:], in0=f2_[:], scalar1=-1.0)
                nc.vector.tensor_max(out=f2_[:], in0=f2_[:], in1=A2[:])
                nc.scalar.activation(out=cos2[:], in_=f2_[:], func=AF.Sin,
                                     scale=-TWO_PI_M, bias=pi_half_t[:D2])

                # temporal cos table
                cost = tb2.tile([MT, NI], DT.bfloat16, tag="cost")
                A3 = ch.tile([MT, NI], DT.float32, tag="Ac")
                f3_ = ch.tile([MT, NI], DT.float32, tag="fc")
                scal3 = sc[:MT, 4:5]
                nc.vector.tensor_scalar(out=A3[:], in0=b_tab[:], scalar1=scal3,
                                        scalar2=MAGIC, op0=OP.mult, op1=OP.add)
                nc.vector.tensor_scalar_add(out=A3[:], in0=A3[:], scalar1=-MAGIC)
                nc.vector.scalar_tensor_tensor(out=f3_[:], in0=b_tab[:], scalar=scal3,
                                               in1=A3[:], op0=OP.mult, op1=OP.subtract)
                nc.vector.tensor_scalar_mul(out=A3[:], in0=f3_[:], scalar1=-1.0)
                nc.vector.tensor_max(out=f3_[:], in0=f3_[:], in1=A3[:])
                nc.scalar.activation(out=cost[:], in_=f3_[:], func=AF.Sin,
                                     scale=-TWO_PI_M, bias=pi_half_t[:MT])

                # matmuls per 512-column chunk; reduced rows accumulate in R
                R = psr.tile([33, NI], DT.float32, tag="red")
                for q in range(NCHUNKS):
                    cs_ = slice(q * NCH, (q + 1) * NCH)
                    u_ps = ps.tile([D2, NCH], DT.float32, tag="u")
                    v_ps = ps.tile([D2, NCH], DT.float32, tag="v")
                    for c in range(3):
                        gcs = slice(c * NI + q * NCH, c * NI + (q + 1) * NCH)
                        nc.tensor.matmul(u_ps[:], wk[:, c * D2:(c + 1) * D2], cos1[:, gcs],
                                         start=(c == 0), stop=(c == 2))
                        nc.tensor.matmul(v_ps[:], wk[:, c * D2:(c + 1) * D2], sin1[:, gcs],
                                         start=(c == 0), stop=(c == 2))
                    t1m = ch.tile([D2, NCH], DT.bfloat16, tag="t1m")
                    t2m = ch.tile([D2, NCH], DT.bfloat16, tag="t2m")
                    nc.vector.tensor_mul(out=t1m[:], in0=sin2[:, cs_], in1=u_ps[:])
                    nc.vector.tensor_mul(out=t2m[:], in0=cos2[:, cs_], in1=v_ps[:])
                    nc.tensor.matmul(R[0:1, cs_], ones121[:], t1m[:], start=True, stop=False)
                    nc.tensor.matmul(R[0:1, cs_], ones121[:], t2m[:], start=False, stop=True)
                    nc.tensor.matmul(R[32:33, cs_], tkw[:], cost[:, cs_], start=True, stop=True)
                # PSUM->SBUF: spatial row on DVE, temporal row on ACT
                nc.vector.tensor_copy(out=stg_sp[0:1, tt_i * NI:(tt_i + 1) * NI], in_=R[0:1, :])
                nc.scalar.copy(out=stg_t[0:1, tt_i * NI:(tt_i + 1) * NI], in_=R[32:33, :])

            # ---------------- tail: point-major combine ----------------
            nc.sync.dma_start(dbg_sp[:], stg_sp[:])
            nc.sync.dma_start(dbg_t[:], stg_t[:])
            spat_pm = pm.tile([128, 128], DT.bfloat16)
            temp_pm = pm.tile([128, 128], DT.bfloat16)
            nc.sync.dma_start(spat_pm[:], dbg_sp[:].rearrange("o (p f) -> (o p) f", p=128))
            nc.sync.dma_start(temp_pm[:], dbg_t[:].rearrange("o (p f) -> (o p) f", p=128))
            spat2 = pm.tile([128, 128], DT.float32)
            nc.vector.tensor_copy(out=spat2[:], in_=spat_pm[:])   # absorb DMA sems
            temp2 = pm.tile([128, 128], DT.float32)
            nc.vector.tensor_copy(out=temp2[:], in_=temp_pm[:])
            nc.vector.tensor_mul(out=spat2[:], in0=spat2[:], in1=rdinv[:])
            nc.vector.tensor_mul(out=temp2[:], in0=temp2[:], in1=env_pm[:])
            nc.vector.tensor_mul(out=spat2[:], in0=spat2[:], in1=temp2[:])
            nc.vector.tensor_add(out=spat2[:], in0=spat2[:], in1=green[:])
            outt = pm.tile([128, 128], DT.float32)
            nc.vector.tensor_mul(out=outt[:], in0=spat2[:], in1=maskout[:])
            nc.sync.dma_start(out_d.rearrange("(p f) -> p f", p=128), outt[:])
    return nc


class SafeTileContext(tile.TileContext):
    """TileContext for a walrus build with tight per-instruction sync-wait
    limits (DMAs: 1, compute: 2). Excess waits are moved onto injected
    single-wait NOPs placed immediately before the instruction on the same
    engine, and the exit drain is split the same way."""

    _WAIT_LIMITS = {"InstDMACopy": 1, "InstDrain": 1, "InstMemSet": 1}
    _DEFAULT_WAIT_LIMIT = 1

    def schedule_and_allocate(self):
        ret = super().schedule_and_allocate()
        nc = self.nc
        eng_obj = {
            mybir.EngineType.PE: nc.tensor,
            mybir.EngineType.DVE: nc.vector,
            mybir.EngineType.Activation: nc.scalar,
            mybir.EngineType.Pool: nc.gpsimd,
            mybir.EngineType.SP: nc.sync,
        }
        # pass 1: collect instructions carrying too many waits
        fixes = []
        for bb in nc.main_func.blocks:
            insts = bb.instructions
            for i, ins in enumerate(insts):
                si = ins.sync_info
                waits = list(si.on_wait) if si and si.on_wait else []
                limit = self._WAIT_LIMITS.get(type(ins).__name__,
                                              self._DEFAULT_WAIT_LIMIT)
                if len(waits) > limit:
                    fixes.append((insts, i, ins, waits, limit))
        # pass 2: apply in reverse index order per list
        for insts, i, ins, waits, limit in sorted(fixes, key=lambda f: -f[1]):
            si = ins.sync_info
            ins.sync_info = mybir.SyncInfo(
                on_wait=waits[-limit:], on_update=list(si.on_update or []))
            at = i
            if (type(ins).__name__ == "InstMatmult" and i > 0
                    and type(insts[i - 1]).__name__ == "InstLdweights"):
                at = i - 1
            for j, w in enumerate(waits[:-limit]):
                nb = eng_obj[ins.engine].nop()
                nop_ins = nb.ins
                # relocate from wherever nop() appended it
                for bb2 in nc.main_func.blocks:
                    if bb2.instructions and bb2.instructions[-1] is nop_ins:
                        bb2.instructions.pop()
                        break
                nop_ins.sync_info = mybir.SyncInfo(on_wait=[w], on_update=[])
                insts.insert(at + j, nop_ins)
        return ret

    def _drain_and_barrier(self, tick_clock, wait_clock):
        nc = self.nc
        nop0 = nc.sync.nop()
        wait_clock.add_sem_waits(nop0.ins, tile.ScopedClock({None: tick_clock.global_clock}))
        waits = list(nop0.ins.sync_info.on_wait or []) if nop0.ins.sync_info else []
        if len(waits) > 1:
            upd = nop0.ins.sync_info.on_update or []
            nop0.ins.sync_info = mybir.SyncInfo(on_wait=[waits[0]], on_update=list(upd))
            for w in waits[1:]:
                nk = nc.sync.nop()
                nk.ins.sync_info = mybir.SyncInfo(on_wait=[w], on_update=[])
        nc.sync.drain()
        nc.all_engine_barrier()
        assert self.sems is not None
        popped = nc._tile_sem_poison_stack.pop()
        assert popped is self._sem_poison
        nc.clear_and_free_semaphores(list(self.sems.allocated().values()))
        nc.all_engine_barrier()


def _host_constants(spatial_kernel, temporal_kernel):
    k = np.asarray(spatial_kernel, dtype=f32)
    K = k.reshape(D2, D1)                       # K[a, b] = k[a*D1 + b]
    wk = np.empty((C1, 3 * D2), dtype=bf16)
    for c in range(3):
        wk[:, c * D2:(c + 1) * D2] = K[:, c * C1:(c + 1) * C1].T.astype(bf16)
    sc = np.zeros((128, 8), dtype=f32)
    p = np.arange(128, dtype=f32)
    sc[:, 0] = p
    sc[:, 1] = 99 + p
    sc[:, 2] = 198 + p
    sc[:, 3] = p
    freqs = ((np.arange(MT, dtype=f32) + f32(1.0)) * f32(0.1)).astype(f32)
    sc[:MT, 4] = (freqs * f32(INV2PI)).astype(f32)
    tkw = np.asarray(temporal_kernel, dtype=f32).reshape(MT, 1).astype(bf16)
    return wk, sc, tkw


def kernel(spacetime_coords, spatial_kernel, temporal_kernel,
           mass_parameter, coupling_strength):
    coords = np.ascontiguousarray(np.asarray(spacetime_coords, dtype=np.float32))
    wk, sc, tkw = _host_constants(spatial_kernel, temporal_kernel)
    nc = _build_nc(float(np.float32(mass_parameter)),
                   float(np.float32(coupling_strength)))
    in_maps = []
    for c in range(N_CORES):
        in_maps.append({
            "coords": np.ascontiguousarray(coords[c * NPT:(c + 1) * NPT]),
            "wk": wk, "sc": sc, "tkw": tkw,
        })
    import os, time
    trace = bool(int(os.environ.get("KERNEL_TRACE", "0")))
    try:
        res = run_bass_kernel_spmd(nc, in_maps, core_ids=list(range(N_CORES)),
                                   trace=trace)
    except ModuleNotFoundError:
        res = run_bass_kernel_spmd(nc, in_maps, core_ids=list(range(N_CORES)))
    if res.exec_time_ns is not None:
        print(f"HW exec time: {res.exec_time_ns} ns")
    return np.concatenate([res.results[c]["out"] for c in range(N_CORES)])


if __name__ == "__main__":
    rng = np.random.default_rng(0)
    ins = {
        "spacetime_coords": (rng.standard_normal((131072, 4)) * 2.0).astype(np.float32),
        "spatial_kernel": (rng.standard_normal(35937) * 0.1).astype(np.float32),
        "temporal_kernel": (rng.standard_normal(33) * 0.1).astype(np.float32),
        "mass_parameter": np.float32(1.0),
        "coupling_strength": np.float32(0.1),
    }
    out = kernel(**ins)
    print("out", out.shape, out.dtype, float(np.abs(out).max()))



# revision 4
# speedup vs baseline: 9.9614x; 9.9614x over previous
"""Causal kernel (nn_CausalKernel) for 8x TRN2 NeuronCores.

Algorithm: sum_n k_n sin(n*r) decomposed via n = a*297 + b:
  sin(n r) = sin_a cos_b + cos_a sin_b with
  sin_b = sin(2pi frac(b * r/2pi)), sin_a = sin(2pi frac(a * 297r/2pi)).
Per-point trig tables are built mode-major ([modes, points]) with a
magic-number round chain feeding the ScalarE Sin LUT (valid range [-pi, pi]);
the 35937-mode contraction runs on TensorE in bf16.

Pure data parallel: 8 cores x 16384 points; weights replicated.

Dispatch: the jitted shard_map executable, the Bass program, and the
device-resident output seed buffers are all built once per process and
cached; each call only uploads the (new) inputs, launches, and fetches the
single output tensor. mass/coupling enter through the sc constant table, so
changing them does not recompile.
"""
import sys
sys.path.insert(0, "/opt/trn_rl_repo")

import numpy as np
import ml_dtypes

import concourse.bass as bass
import concourse.mybir as mybir
import concourse.tile as tile

f32 = np.float32
bf16 = ml_dtypes.bfloat16

N_CORES = 8
NPT = 16384            # points per core
NI = 2048              # points per point-tile
NTILES = NPT // NI     # 8
NCH = 512              # matmul moving-dim chunk (one PSUM bank)
NCHUNKS = NI // NCH    # 4

D1, D2 = 297, 121      # n = a*D1 + b
C1 = 99                # D1 contraction chunk rows (3 chunks)
MT = 33                # temporal modes

MAGIC = float(f32(1.5 * 2 ** 23))
INV2PI = float(f32(1.0 / (2 * np.pi)))
TWO_PI_M = float(f32(6.2831845))   # < 2pi so |scale*0.5| <= pi
PI_HALF = float(f32(np.pi / 2))
DT = mybir.dt


def _build_nc(use_act_offload: bool = True):
    nc = bass.Bass(target_bir_lowering=False)
    AF = mybir.ActivationFunctionType
    OP = mybir.AluOpType

    coords_in = nc.dram_tensor("coords", [NPT, 4], DT.float32, kind="ExternalInput")
    wk_in = nc.dram_tensor("wk", [C1, 3 * D2], DT.bfloat16, kind="ExternalInput")
    sc_in = nc.dram_tensor("sc", [128, 8], DT.float32, kind="ExternalInput")
    tkw_in = nc.dram_tensor("tkw", [MT, 1], DT.bfloat16, kind="ExternalInput")
    out_d = nc.dram_tensor("out", [NPT], DT.float32, kind="ExternalOutput")
    stg_sp_d = nc.dram_tensor("stg_sp", [1, NPT], DT.bfloat16)
    stg_t_d = nc.dram_tensor("stg_t", [1, NPT], DT.bfloat16)
    bpsi_d = nc.dram_tensor("bpsi", [1, NPT], DT.float32)
    bphi_d = nc.dram_tensor("bphi", [1, NPT], DT.float32)
    btab_d = nc.dram_tensor("btab", [1, NPT], DT.float32)

    with SafeTileContext(nc) as tc:
        with (
            tc.tile_pool(name="const", bufs=1) as cpool,
            tc.tile_pool(name="pm", bufs=1) as pm,          # point-major persistents
            tc.tile_pool(name="bc", bufs=2) as bc,          # broadcast tiles
            tc.tile_pool(name="chain", bufs=2) as ch,       # chain scratch
            tc.tile_pool(name="tab", bufs=2) as tb,
            tc.tile_pool(name="tab2", bufs=1) as tb2,         # bf16 tables
            tc.tile_pool(name="ps", bufs=2, space="PSUM") as ps,
            tc.tile_pool(name="psr", bufs=1, space="PSUM") as psr,
        ):
            # ---------------- constants ----------------
            sc0 = cpool.tile([128, 8], DT.float32)
            nc.sync.dma_start(sc0[:], sc_in[:])
            sc = cpool.tile([128, 8], DT.float32)
            nc.vector.tensor_copy(out=sc[:], in_=sc0[:])    # absorb DMA sem on DVE

            wk0 = cpool.tile([C1, 3 * D2], DT.bfloat16)
            nc.sync.dma_start(wk0[:], wk_in[:])
            wk = cpool.tile([C1, 3 * D2], DT.bfloat16)
            nc.vector.tensor_copy(out=wk[:], in_=wk0[:])

            tkw0 = cpool.tile([MT, 1], DT.bfloat16)
            nc.sync.dma_start(tkw0[:], tkw_in[:])
            tkw = cpool.tile([MT, 1], DT.bfloat16)
            nc.vector.tensor_copy(out=tkw[:], in_=tkw0[:])

            ones121 = cpool.tile([D2, 1], DT.bfloat16)
            nc.vector.memset(ones121[:], 1.0)
            pi_half_t = cpool.tile([128, 1], DT.float32)
            nc.vector.memset(pi_half_t[:], PI_HALF)
            magic_t = cpool.tile([128, 1], DT.float32)
            nc.vector.memset(magic_t[:], MAGIC)
            nmagic_t = cpool.tile([128, 1], DT.float32)
            nc.vector.memset(nmagic_t[:], -MAGIC)

            # ---------------- stage 0: point-major precompute ----------------
            crd = pm.tile([128, 512], DT.float32)
            nc.sync.dma_start(crd[:], coords_in.rearrange("(p f) c -> p (f c)", p=128))
            crd4 = crd[:].rearrange("p (f c) -> p f c", c=4)

            t_pm = pm.tile([128, 128], DT.float32)
            nc.vector.tensor_copy(out=t_pm[:], in_=crd4[:, :, 0])
            xx = pm.tile([128, 128], DT.float32, tag="w1")
            yy = pm.tile([128, 128], DT.float32, tag="w2")
            zz = pm.tile([128, 128], DT.float32, tag="w3")
            nc.vector.tensor_mul(out=xx[:], in0=crd4[:, :, 1], in1=crd4[:, :, 1])
            nc.vector.tensor_mul(out=yy[:], in0=crd4[:, :, 2], in1=crd4[:, :, 2])
            nc.vector.tensor_mul(out=zz[:], in0=crd4[:, :, 3], in1=crd4[:, :, 3])
            sdsq = pm.tile([128, 128], DT.float32)
            nc.vector.tensor_add(out=sdsq[:], in0=xx[:], in1=yy[:])
            nc.vector.tensor_add(out=sdsq[:], in0=sdsq[:], in1=zz[:])
            r2e = pm.tile([128, 128], DT.float32)
            nc.vector.tensor_scalar_add(out=r2e[:], in0=sdsq[:], scalar1=float(f32(1e-12)))

            # r = sqrt(r2e) with two Newton refinements (HW sqrt LUT is loose)
            r_pm = pm.tile([128, 128], DT.float32)
            nc.scalar.activation(out=r_pm[:], in_=r2e[:], func=AF.Sqrt)
            tmpa = pm.tile([128, 128], DT.float32, tag="w1")
            tmpb = pm.tile([128, 128], DT.float32, tag="w2")
            for _ in range(2):
                nc.vector.reciprocal(out=tmpa[:], in_=r_pm[:])
                nc.vector.tensor_mul(out=tmpb[:], in0=r2e[:], in1=tmpa[:])
                nc.vector.tensor_add(out=tmpb[:], in0=tmpb[:], in1=r_pm[:])
                nc.vector.tensor_scalar_mul(out=r_pm[:], in0=tmpb[:], scalar1=0.5)

            # psi1 = frac(r/2pi), signed
            A0 = pm.tile([128, 128], DT.float32)
            m0 = pm.tile([128, 128], DT.float32)
            psi1 = pm.tile([128, 128], DT.float32)
            nc.vector.tensor_scalar(out=A0[:], in0=r_pm[:], scalar1=INV2PI,
                                    scalar2=MAGIC, op0=OP.mult, op1=OP.add)
            nc.vector.tensor_scalar_add(out=m0[:], in0=A0[:], scalar1=-MAGIC)
            nc.vector.scalar_tensor_tensor(out=psi1[:], in0=r_pm[:], scalar=INV2PI,
                                           in1=m0[:], op0=OP.mult, op1=OP.subtract)

            # phi1 = frac(D1 * r / 2pi) via 12-bit split of r (accuracy for a<=120 amplification)
            SC12 = float(f32(2.0 ** 12))
            c2_64 = np.float64(D1) / (2 * np.pi)
            c2h = float(f32(np.trunc(c2_64 * 2 ** 12) / 2 ** 12))
            c2l = float(f32(c2_64 - np.float64(f32(c2h))))
            c2f = float(f32(c2_64))
            rh = pm.tile([128, 128], DT.float32)
            rl = pm.tile([128, 128], DT.float32)
            nc.vector.tensor_scalar(out=A0[:], in0=r_pm[:], scalar1=SC12,
                                    scalar2=MAGIC, op0=OP.mult, op1=OP.add)
            nc.vector.tensor_scalar_add(out=m0[:], in0=A0[:], scalar1=-MAGIC)
            nc.vector.tensor_scalar_mul(out=rh[:], in0=m0[:], scalar1=float(f32(2.0 ** -12)))
            nc.vector.tensor_sub(out=rl[:], in0=r_pm[:], in1=rh[:])
            # t1 = rh*c2h (exact); f1 = frac(t1)
            t1t = pm.tile([128, 128], DT.float32, tag="w3")
            nc.vector.tensor_scalar(out=A0[:], in0=rh[:], scalar1=c2h,
                                    scalar2=MAGIC, op0=OP.mult, op1=OP.add)
            nc.vector.tensor_scalar_add(out=m0[:], in0=A0[:], scalar1=-MAGIC)
            nc.vector.scalar_tensor_tensor(out=t1t[:], in0=rh[:], scalar=c2h,
                                           in1=m0[:], op0=OP.mult, op1=OP.subtract)
            # rest = rh*c2l + rl*c2 ; ph = f1 + rest ; phi1 = frac(ph)
            nc.vector.tensor_scalar_mul(out=tmpa[:], in0=rl[:], scalar1=c2f)
            nc.vector.scalar_tensor_tensor(out=tmpb[:], in0=rh[:], scalar=c2l,
                                           in1=tmpa[:], op0=OP.mult, op1=OP.add)
            ph_t = pm.tile([128, 128], DT.float32)
            nc.vector.tensor_add(out=ph_t[:], in0=t1t[:], in1=tmpb[:])
            phi1 = pm.tile([128, 128], DT.float32)
            nc.vector.tensor_scalar(out=A0[:], in0=ph_t[:], scalar1=1.0,
                                    scalar2=MAGIC, op0=OP.mult, op1=OP.add)
            nc.vector.tensor_scalar_add(out=m0[:], in0=A0[:], scalar1=-MAGIC)
            nc.vector.tensor_sub(out=phi1[:], in0=ph_t[:], in1=m0[:])

            # |t|, envelope, green, masks, 1/(r+1e-6)
            tabs = pm.tile([128, 128], DT.float32)
            nc.vector.tensor_scalar_mul(out=tabs[:], in0=t_pm[:], scalar1=-1.0)
            nc.vector.tensor_max(out=tabs[:], in0=tabs[:], in1=t_pm[:])
            env_pm = pm.tile([128, 128], DT.float32)
            nc.scalar.activation(out=env_pm[:], in_=tabs[:], func=AF.Exp,
                                 scale=float(f32(-0.1)))
            # exp(-mp * r): -mp comes in via sc column 5 (per-partition scale)
            expg = pm.tile([128, 128], DT.float32)
            nc.scalar.activation(out=expg[:], in_=r_pm[:], func=AF.Exp,
                                 scale=sc[:, 5:6])
            rinv = pm.tile([128, 128], DT.float32)
            nc.vector.reciprocal(out=rinv[:], in_=r_pm[:])
            green = pm.tile([128, 128], DT.float32)
            nc.vector.tensor_mul(out=green[:], in0=expg[:], in1=rinv[:])
            # * coupling_strength via sc column 6
            nc.vector.tensor_scalar_mul(out=green[:], in0=green[:], scalar1=sc[:, 6:7])
            rden = pm.tile([128, 128], DT.float32)
            nc.vector.tensor_scalar_add(out=rden[:], in0=r_pm[:], scalar1=float(f32(1e-6)))
            rdinv = pm.tile([128, 128], DT.float32)
            nc.vector.reciprocal(out=rdinv[:], in_=rden[:])

            tsq = pm.tile([128, 128], DT.float32)
            nc.vector.tensor_mul(out=tsq[:], in0=t_pm[:], in1=t_pm[:])
            interval = pm.tile([128, 128], DT.float32)
            nc.vector.tensor_sub(out=interval[:], in0=tsq[:], in1=sdsq[:])
            mg1 = pm.tile([128, 128], DT.float32, tag="w4")
            mg2 = pm.tile([128, 128], DT.float32, tag="w5")
            nc.vector.tensor_scalar(out=mg1[:], in0=interval[:], scalar1=0.0,
                                    scalar2=None, op0=OP.is_gt)
            nc.vector.tensor_scalar(out=mg2[:], in0=t_pm[:], scalar1=0.0,
                                    scalar2=None, op0=OP.is_gt)
            nc.vector.tensor_mul(out=mg1[:], in0=mg1[:], in1=mg2[:])
            nc.vector.tensor_mul(out=green[:], in0=green[:], in1=mg1[:])
            mo1 = pm.tile([128, 128], DT.float32, tag="w4")
            mo2 = pm.tile([128, 128], DT.float32, tag="w5")
            nc.vector.tensor_scalar(out=mo1[:], in0=interval[:], scalar1=0.0,
                                    scalar2=None, op0=OP.is_ge)
            nc.vector.tensor_scalar(out=mo2[:], in0=t_pm[:], scalar1=0.0,
                                    scalar2=None, op0=OP.is_ge)
            maskout = pm.tile([128, 128], DT.float32)
            nc.vector.tensor_mul(out=maskout[:], in0=mo1[:], in1=mo2[:])

            # bases to DRAM for broadcast-DMA sourcing
            nc.sync.dma_start(bpsi_d[:].rearrange("o (p f) -> (o p) f", p=128), psi1[:])
            nc.sync.dma_start(bphi_d[:].rearrange("o (p f) -> (o p) f", p=128), phi1[:])
            nc.sync.dma_start(btab_d[:].rearrange("o (p f) -> (o p) f", p=128), tabs[:])

            # staging for reduced rows (bf16)
            stg_sp = pm.tile([1, NPT], DT.bfloat16)
            stg_t = pm.tile([1, NPT], DT.bfloat16)

            # ---------------- per point-tile mode-major pipeline ----------------
            for tt_i in range(NTILES):
                pslc = slice(tt_i * NI, (tt_i + 1) * NI)
                b_psi = bc.tile([C1, NI], DT.float32, tag="b_psi")
                b_phi = bc.tile([D2, NI], DT.float32, tag="b_phi")
                b_tab = bc.tile([MT, NI], DT.float32, tag="b_tab")
                nc.gpsimd.dma_start(b_psi[:], bpsi_d[0:1, pslc].to_broadcast((C1, NI)))
                nc.gpsimd.dma_start(b_phi[:], bphi_d[0:1, pslc].to_broadcast((D2, NI)))
                nc.gpsimd.dma_start(b_tab[:], btab_d[0:1, pslc].to_broadcast((MT, NI)))

                # D1 tables: f in [-0.5,0.5]; sin/cos via Sin LUT; bf16 out
                sin1 = tb2.tile([C1, 3 * NI], DT.bfloat16, tag="sin1")
                cos1 = tb2.tile([C1, 3 * NI], DT.bfloat16, tag="cos1")
                for c in range(3):
                    Ac = ch.tile([C1, NI], DT.float32, tag="Ac")
                    fc_ = ch.tile([C1, NI], DT.float32, tag="fc")
                    scal = sc[:C1, c:c + 1]
                    if use_act_offload:
                        nc.scalar.activation(out=Ac[:], in_=b_psi[:], func=AF.Identity,
                                             bias=magic_t[:C1], scale=scal)
                        nc.scalar.activation(out=Ac[:], in_=Ac[:], func=AF.Identity,
                                             bias=nmagic_t[:C1], scale=1.0)
                    else:
                        nc.vector.tensor_scalar(out=Ac[:], in0=b_psi[:], scalar1=scal,
                                                scalar2=MAGIC, op0=OP.mult, op1=OP.add)
                        nc.vector.tensor_scalar_add(out=Ac[:], in0=Ac[:], scalar1=-MAGIC)
                    nc.vector.scalar_tensor_tensor(out=fc_[:], in0=b_psi[:], scalar=scal,
                                                   in1=Ac[:], op0=OP.mult, op1=OP.subtract)
                    nc.scalar.activation(out=sin1[:, c * NI:(c + 1) * NI], in_=fc_[:],
                                         func=AF.Sin, scale=TWO_PI_M)
                    nc.vector.tensor_scalar_mul(out=Ac[:], in0=fc_[:], scalar1=-1.0)
                    nc.vector.tensor_max(out=fc_[:], in0=fc_[:], in1=Ac[:])
                    nc.scalar.activation(out=cos1[:, c * NI:(c + 1) * NI], in_=fc_[:],
                                         func=AF.Sin, scale=-TWO_PI_M, bias=pi_half_t[:C1])

                # D2 tables
                sin2 = tb2.tile([D2, NI], DT.bfloat16, tag="sin2")
                cos2 = tb2.tile([D2, NI], DT.bfloat16, tag="cos2")
                A2 = ch.tile([D2, NI], DT.float32, tag="Ac")
                f2_ = ch.tile([D2, NI], DT.float32, tag="fc")
                scal2 = sc[:D2, 3:4]
                nc.vector.tensor_scalar(out=A2[:], in0=b_phi[:], scalar1=scal2,
                                        scalar2=MAGIC, op0=OP.mult, op1=OP.add)
                nc.vector.tensor_scalar_add(out=A2[:], in0=A2[:], scalar1=-MAGIC)
                nc.vector.scalar_tensor_tensor(out=f2_[:], in0=b_phi[:], scalar=scal2,
                                               in1=A2[:], op0=OP.mult, op1=OP.subtract)
                nc.scalar.activation(out=sin2[:], in_=f2_[:], func=AF.Sin, scale=TWO_PI_M)
                nc.vector.tensor_scalar_mul(out=A2[:], in0=f2_[:], scalar1=-1.0)
                nc.vector.tensor_max(out=f2_[:], in0=f2_[:], in1=A2[:])
                nc.scalar.activation(out=cos2[:], in_=f2_[:], func=AF.Sin,
                                     scale=-TWO_PI_M, bias=pi_half_t[:D2])

                # temporal cos table
                cost = tb2.tile([MT, NI], DT.bfloat16, tag="cost")
                A3 = ch.tile([MT, NI], DT.float32, tag="Ac")
                f3_ = ch.tile([MT, NI], DT.float32, tag="fc")
                scal3 = sc[:MT, 4:5]
                nc.vector.tensor_scalar(out=A3[:], in0=b_tab[:], scalar1=scal3,
                                        scalar2=MAGIC, op0=OP.mult, op1=OP.add)
                nc.vector.tensor_scalar_add(out=A3[:], in0=A3[:], scalar1=-MAGIC)
                nc.vector.scalar_tensor_tensor(out=f3_[:], in0=b_tab[:], scalar=scal3,
                                               in1=A3[:], op0=OP.mult, op1=OP.subtract)
                nc.vector.tensor_scalar_mul(out=A3[:], in0=f3_[:], scalar1=-1.0)
                nc.vector.tensor_max(out=f3_[:], in0=f3_[:], in1=A3[:])
                nc.scalar.activation(out=cost[:], in_=f3_[:], func=AF.Sin,
                                     scale=-TWO_PI_M, bias=pi_half_t[:MT])

                # matmuls per 512-column chunk; reduced rows accumulate in R
                R = psr.tile([33, NI], DT.float32, tag="red")
                for q in range(NCHUNKS):
                    cs_ = slice(q * NCH, (q + 1) * NCH)
                    u_ps = ps.tile([D2, NCH], DT.float32, tag="u")
                    v_ps = ps.tile([D2, NCH], DT.float32, tag="v")
                    for c in range(3):
                        gcs = slice(c * NI + q * NCH, c * NI + (q + 1) * NCH)
                        nc.tensor.matmul(u_ps[:], wk[:, c * D2:(c + 1) * D2], cos1[:, gcs],
                                         start=(c == 0), stop=(c == 2))
                        nc.tensor.matmul(v_ps[:], wk[:, c * D2:(c + 1) * D2], sin1[:, gcs],
                                         start=(c == 0), stop=(c == 2))
                    t1m = ch.tile([D2, NCH], DT.bfloat16, tag="t1m")
                    t2m = ch.tile([D2, NCH], DT.bfloat16, tag="t2m")
                    nc.vector.tensor_mul(out=t1m[:], in0=sin2[:, cs_], in1=u_ps[:])
                    nc.vector.tensor_mul(out=t2m[:], in0=cos2[:, cs_], in1=v_ps[:])
                    nc.tensor.matmul(R[0:1, cs_], ones121[:], t1m[:], start=True, stop=False)
                    nc.tensor.matmul(R[0:1, cs_], ones121[:], t2m[:], start=False, stop=True)
                    nc.tensor.matmul(R[32:33, cs_], tkw[:], cost[:, cs_], start=True, stop=True)
                # PSUM->SBUF: spatial row on DVE, temporal row on ACT
                nc.vector.tensor_copy(out=stg_sp[0:1, tt_i * NI:(tt_i + 1) * NI], in_=R[0:1, :])
                nc.scalar.copy(out=stg_t[0:1, tt_i * NI:(tt_i + 1) * NI], in_=R[32:33, :])

            # ---------------- tail: point-major combine ----------------
            nc.sync.dma_start(stg_sp_d[:], stg_sp[:])
            nc.sync.dma_start(stg_t_d[:], stg_t[:])
            spat_pm = pm.tile([128, 128], DT.bfloat16)
            temp_pm = pm.tile([128, 128], DT.bfloat16)
            nc.sync.dma_start(spat_pm[:], stg_sp_d[:].rearrange("o (p f) -> (o p) f", p=128))
            nc.sync.dma_start(temp_pm[:], stg_t_d[:].rearrange("o (p f) -> (o p) f", p=128))
            spat2 = pm.tile([128, 128], DT.float32)
            nc.vector.tensor_copy(out=spat2[:], in_=spat_pm[:])   # absorb DMA sems
            temp2 = pm.tile([128, 128], DT.float32)
            nc.vector.tensor_copy(out=temp2[:], in_=temp_pm[:])
            nc.vector.tensor_mul(out=spat2[:], in0=spat2[:], in1=rdinv[:])
            nc.vector.tensor_mul(out=temp2[:], in0=temp2[:], in1=env_pm[:])
            nc.vector.tensor_mul(out=spat2[:], in0=spat2[:], in1=temp2[:])
            nc.vector.tensor_add(out=spat2[:], in0=spat2[:], in1=green[:])
            outt = pm.tile([128, 128], DT.float32)
            nc.vector.tensor_mul(out=outt[:], in0=spat2[:], in1=maskout[:])
            nc.sync.dma_start(out_d.rearrange("(p f) -> p f", p=128), outt[:])
    return nc


class SafeTileContext(tile.TileContext):
    """TileContext for a walrus build with tight per-instruction sync-wait
    limits (DMAs: 1, compute: 2). Excess waits are moved onto injected
    single-wait NOPs placed immediately before the instruction on the same
    engine, and the exit drain is split the same way."""

    _WAIT_LIMITS = {"InstDMACopy": 1, "InstDrain": 1, "InstMemSet": 1}
    _DEFAULT_WAIT_LIMIT = 1

    def schedule_and_allocate(self):
        ret = super().schedule_and_allocate()
        nc = self.nc
        eng_obj = {
            mybir.EngineType.PE: nc.tensor,
            mybir.EngineType.DVE: nc.vector,
            mybir.EngineType.Activation: nc.scalar,
            mybir.EngineType.Pool: nc.gpsimd,
            mybir.EngineType.SP: nc.sync,
        }
        # pass 1: collect instructions carrying too many waits
        fixes = []
        for bb in nc.main_func.blocks:
            insts = bb.instructions
            for i, ins in enumerate(insts):
                si = ins.sync_info
                waits = list(si.on_wait) if si and si.on_wait else []
                limit = self._WAIT_LIMITS.get(type(ins).__name__,
                                              self._DEFAULT_WAIT_LIMIT)
                if len(waits) > limit:
                    fixes.append((insts, i, ins, waits, limit))
        # pass 2: apply in reverse index order per list
        for insts, i, ins, waits, limit in sorted(fixes, key=lambda f: -f[1]):
            si = ins.sync_info
            ins.sync_info = mybir.SyncInfo(
                on_wait=waits[-limit:], on_update=list(si.on_update or []))
            at = i
            if (type(ins).__name__ == "InstMatmult" and i > 0
                    and type(insts[i - 1]).__name__ == "InstLdweights"):
                at = i - 1
            for j, w in enumerate(waits[:-limit]):
                nb = eng_obj[ins.engine].nop()
                nop_ins = nb.ins
                # relocate from wherever nop() appended it
                for bb2 in nc.main_func.blocks:
                    if bb2.instructions and bb2.instructions[-1] is nop_ins:
                        bb2.instructions.pop()
                        break
                nop_ins.sync_info = mybir.SyncInfo(on_wait=[w], on_update=[])
                insts.insert(at + j, nop_ins)
        return ret

    def _drain_and_barrier(self, tick_clock, wait_clock):
        nc = self.nc
        nop0 = nc.sync.nop()
        wait_clock.add_sem_waits(nop0.ins, tile.ScopedClock({None: tick_clock.global_clock}))
        waits = list(nop0.ins.sync_info.on_wait or []) if nop0.ins.sync_info else []
        if len(waits) > 1:
            upd = nop0.ins.sync_info.on_update or []
            nop0.ins.sync_info = mybir.SyncInfo(on_wait=[waits[0]], on_update=list(upd))
            for w in waits[1:]:
                nk = nc.sync.nop()
                nk.ins.sync_info = mybir.SyncInfo(on_wait=[w], on_update=[])
        nc.sync.drain()
        nc.all_engine_barrier()
        assert self.sems is not None
        popped = nc._tile_sem_poison_stack.pop()
        assert popped is self._sem_poison
        nc.clear_and_free_semaphores(list(self.sems.allocated().values()))
        nc.all_engine_barrier()


def _host_constants(spatial_kernel, temporal_kernel, mass_parameter, coupling_strength):
    k = np.asarray(spatial_kernel, dtype=f32)
    K = k.reshape(D2, D1)                       # K[a, b] = k[a*D1 + b]
    wk = np.empty((C1, 3 * D2), dtype=bf16)
    for c in range(3):
        wk[:, c * D2:(c + 1) * D2] = K[:, c * C1:(c + 1) * C1].T.astype(bf16)
    sc = np.zeros((128, 8), dtype=f32)
    p = np.arange(128, dtype=f32)
    sc[:, 0] = p
    sc[:, 1] = 99 + p
    sc[:, 2] = 198 + p
    sc[:, 3] = p
    freqs = ((np.arange(MT, dtype=f32) + f32(1.0)) * f32(0.1)).astype(f32)
    sc[:MT, 4] = (freqs * f32(INV2PI)).astype(f32)
    sc[:, 5] = -f32(mass_parameter)
    sc[:, 6] = f32(coupling_strength)
    tkw = np.asarray(temporal_kernel, dtype=f32).reshape(MT, 1).astype(bf16)
    return wk, sc, tkw


_STATE = None


def _get_state():
    global _STATE
    if _STATE is not None:
        return _STATE

    import jax
    from jax.sharding import Mesh, PartitionSpec, NamedSharding
    import warnings
    with warnings.catch_warnings():
        warnings.simplefilter("ignore")
        try:
            from jax.experimental.shard_map import shard_map
            _rep_kw = "check_rep"
        except ImportError:
            from jax import shard_map
            _rep_kw = "check_vma"
    from concourse import bass2jax

    nc = _build_nc()
    bass2jax.install_neuronx_cc_hook()
    partition_name = nc.partition_id_tensor.name if nc.partition_id_tensor else None
    in_names, out_names, out_avals = [], [], []
    for alloc in nc.m.functions[0].allocations:
        if not isinstance(alloc, mybir.MemoryLocationSet):
            continue
        name = alloc.memorylocations[0].name
        if alloc.kind == "ExternalInput":
            if name != partition_name:
                in_names.append(name)
        elif alloc.kind == "ExternalOutput":
            out_names.append(name)
            out_avals.append(jax.core.ShapedArray(
                tuple(alloc.tensor_shape), mybir.dt.np(alloc.dtype)))
    n_params = len(in_names)
    n_outs = len(out_avals)
    in_names_all = in_names + out_names + ([partition_name] if partition_name else [])

    def _body(*args):
        operands = list(args)
        if partition_name is not None:
            operands.append(bass2jax.partition_id_tensor())
        outs = bass2jax._bass_exec_p.bind(
            *operands, out_avals=tuple(out_avals), in_names=tuple(in_names_all),
            out_names=tuple(out_names), lowering_input_output_aliases=(),
            sim_require_finite=True, sim_require_nnan=True, nc=nc)
        # NB: must return ALL custom-call results — returning a subset
        # desyncs the axon worker.
        return tuple(outs)

    devices = jax.devices()[:N_CORES]
    mesh = Mesh(np.asarray(devices), ("core",))
    sharded = jax.jit(
        shard_map(_body, mesh=mesh,
                  in_specs=(PartitionSpec("core"),) * (n_params + n_outs),
                  out_specs=(PartitionSpec("core"),) * n_outs,
                  **{_rep_kw: False}),
        keep_unused=True)
    sh = NamedSharding(mesh, PartitionSpec("core"))
    # Output seed buffers live on device for the life of the process. The
    # kernel fully overwrites every output element, so their (possibly
    # stale) contents never leak into results; no donation, so XLA never
    # frees them.
    dev_zeros = [
        jax.device_put(np.zeros((N_CORES * av.shape[0], *av.shape[1:]), av.dtype), sh)
        for av in out_avals
    ]
    for z in dev_zeros:
        z.block_until_ready()
    _STATE = dict(sharded=sharded, sh=sh, in_names=in_names,
                  dev_zeros=dev_zeros, jax=jax)
    return _STATE


def kernel(spacetime_coords, spatial_kernel, temporal_kernel,
           mass_parameter, coupling_strength):
    st = _get_state()
    jax = st["jax"]
    coords = np.ascontiguousarray(np.asarray(spacetime_coords, dtype=np.float32))
    wk, sc, tkw = _host_constants(spatial_kernel, temporal_kernel,
                                  mass_parameter, coupling_strength)
    reps = {
        "coords": coords,                       # [8*NPT, 4], sharded by rows
        "wk": np.tile(wk, (N_CORES, 1)),        # replicated per core
        "sc": np.tile(sc, (N_CORES, 1)),
        "tkw": np.tile(tkw, (N_CORES, 1)),
    }
    dev_in = [jax.device_put(reps[n], st["sh"]) for n in st["in_names"]]
    res = st["sharded"](*dev_in, *st["dev_zeros"])
    out = np.asarray(res[0])
    return out.reshape(-1)


if __name__ == "__main__":
    rng = np.random.default_rng(0)
    ins = {
        "spacetime_coords": (rng.standard_normal((131072, 4)) * 2.0).astype(np.float32),
        "spatial_kernel": (rng.standard_normal(35937) * 0.1).astype(np.float32),
        "temporal_kernel": (rng.standard_normal(33) * 0.1).astype(np.float32),
        "mass_parameter": np.float32(1.0),
        "coupling_strength": np.float32(0.1),
    }
    out = kernel(**ins)
    print("out", out.shape, out.dtype, float(np.abs(out).max()))
